# revision 1
# baseline (speedup 1.0000x reference)
"""NeuronPool (moe_routing) Trainium2 kernel.

Expert-parallel over 8 NeuronCores: core c computes neurons [8c, 8c+8) for the
full batch, host concatenates along the neuron axis.

Per-core pipeline (all shapes per core):
  x = [proj | hist_broadcast]  (built on device, stored transposed as 18
      [128,32] f32r tiles so the batch stays on the PSUM partition dim)
  A(n), per neuron:
      psum1[32,512] = sel(n).T @ b1_rows  +  sum_k xT[k].T @ W1[n,k]   (f32r;
          biases/gamma/beta live one-neuron-per-partition and broadcast via a
          K=8 one-hot selector matmul)
      h1 = gelu(psum1)                 -> PE-transpose -> h1T [128,32] x4
      psum2[32,512] = bias + sum_j h1T[j].T @ W2[n,j]
      h2 = gelu(psum2)                 -> PE-transpose -> h2T
      psum3[32,256] = bias + sum_j h2T[j].T @ W3[n,j]
      y = copy(psum3) + row sums (ACT accum_out); yc = y - mean; ssq(yc)
  B(n), emitted one neuron behind A so it pipelines instead of trailing:
      inv_std = 1/sqrt(ssq/D + eps); out = yc*inv_std*(gamma*mod) + beta*mod
The last two neurons' weight DMAs interleave with the layer pipeline so the
final arriving bytes (W3 of the last neuron) feed the shortest compute chain.
Weights stream HBM->SBUF as ~1MiB SWDGE DMAs with an inline fp32->float32r
cast (float32r matmuls run at 4x the fp32 rate; ~1.5e-4 relative rounding).
Measured: 156.3 us HW exec per core, relative error 2.7e-4 vs fp32 reference.
"""
import math
import numpy as np
from contextlib import ExitStack

import concourse.bass as bass
import concourse.tile as tile
from concourse import bacc, mybir
from concourse.bass_utils import run_bass_kernel_spmd

N_CORES = 8
B = 32          # batch
D = 256         # model dim
HIST = 8
HID = 512
N_NEURONS = 64
NPC = N_NEURONS // N_CORES  # 8 neurons per core
IN_DIM = D * (1 + HIST)     # 2304
KC1 = IN_DIM // 128         # 18 contraction chunks for GEMM1
KC2 = HID // 128            # 4 chunks for GEMM2/GEMM3
LN_EPS = 1e-5
FMIN, FMAX = 0.5, 40.0
TICK_INTERVAL = 0.1

f32 = mybir.dt.float32
f32r = mybir.dt.float32r

# packed per-neuron row layout (columns in bvec8: one SBUF partition per
# neuron, broadcast into PSUM via a K=8 one-hot selector matmul)
B1_OFF = 0
B2_OFF = B1_OFF + HID
B3_OFF = B2_OFF + HID
GM_OFF = B3_OFF + D
BM_OFF = GM_OFF + D
BVEC_LEN = BM_OFF + D

_CACHE = {}


def _build_program():
    nc = bacc.Bacc("TRN2", target_bir_lowering=False, debug=False,
                   num_devices=N_CORES)

    emb = nc.dram_tensor("emb", [B, D], f32, kind="ExternalInput").ap()
    wp = nc.dram_tensor("wp", [D, D], f32, kind="ExternalInput").ap()
    bpd = nc.dram_tensor("bpd", [128, 2], f32, kind="ExternalInput").ap()
    histd = nc.dram_tensor("histd", [16, 128], f32, kind="ExternalInput").ap()
    eyed = nc.dram_tensor("eyed", [32, 32], f32, kind="ExternalInput").ap()
    w1d = nc.dram_tensor("w1d", [NPC, 128, KC1, HID], f32, kind="ExternalInput").ap()
    w2d = nc.dram_tensor("w2d", [NPC, 128, KC2, HID], f32, kind="ExternalInput").ap()
    w3d = nc.dram_tensor("w3d", [NPC, 128, KC2, D], f32, kind="ExternalInput").ap()
    bvecd = nc.dram_tensor("bvecd", [NPC, BVEC_LEN], f32, kind="ExternalInput").ap()
    sel8d = nc.dram_tensor("sel8d", [NPC, NPC * B], f32, kind="ExternalInput").ap()
    out = nc.dram_tensor("out", [B, NPC, D], f32, kind="ExternalOutput").ap()

    GELU = mybir.ActivationFunctionType.Gelu
    COPY = mybir.ActivationFunctionType.Copy
    SQUARE = mybir.ActivationFunctionType.Square
    SQRT = mybir.ActivationFunctionType.Sqrt

    with tile.TileContext(nc) as tc, ExitStack() as ctx:
        # SBUF pools
        cst = ctx.enter_context(tc.tile_pool(name="cst", bufs=1))
        xtp = ctx.enter_context(tc.tile_pool(name="xtp", bufs=KC1))
        w1p = ctx.enter_context(tc.tile_pool(name="w1p", bufs=8))
        w23p = ctx.enter_context(tc.tile_pool(name="w23p", bufs=6))
        htp = ctx.enter_context(tc.tile_pool(name="htp", bufs=16))
        hp = ctx.enter_context(tc.tile_pool(name="hp", bufs=4))
        ysp = ctx.enter_context(tc.tile_pool(name="ysp", bufs=NPC))
        rsp = ctx.enter_context(tc.tile_pool(name="rsp", bufs=NPC))
        yp = ctx.enter_context(tc.tile_pool(name="yp", bufs=10))
        stp = ctx.enter_context(tc.tile_pool(name="stp", bufs=12))
        # PSUM pools (8 banks total: 3 + 3 + 2)
        accp = ctx.enter_context(tc.tile_pool(name="accp", bufs=3, space="PSUM"))
        trp = ctx.enter_context(tc.tile_pool(name="trp", bufs=3, space="PSUM"))
        gbp = ctx.enter_context(tc.tile_pool(name="gbp", bufs=2, space="PSUM"))

        # ---- constants ----
        eye = cst.tile([32, 32], f32, tag="eye")
        nc.sync.dma_start(out=eye[:], in_=eyed)
        onesf = cst.tile([1, 32], f32, tag="onesf")
        nc.vector.memset(onesf[:], 1.0)
        onesr = cst.tile([1, 32], f32r, tag="onesr")
        nc.vector.tensor_copy(onesr[:], onesf[:])
        onesb = cst.tile([128, 32], f32, tag="onesb")
        nc.vector.memset(onesb[:], 1.0)
        epst = cst.tile([B, 1], f32, tag="epst")
        nc.vector.memset(epst[:], LN_EPS)
        bpt = cst.tile([128, 2], f32, tag="bpt")
        nc.sync.dma_start(out=bpt[:], in_=bpd)
        bvec = cst.tile([NPC, BVEC_LEN], f32r, tag="bvec")
        nc.gpsimd.dma_start(out=bvec[:], in_=bvecd)
        sel8 = cst.tile([NPC, NPC * B], f32r, tag="sel8")
        nc.gpsimd.dma_start(out=sel8[:], in_=sel8d)

        # K=8 one-hot selector: sel8[:, 32n:32n+32].T @ bvec[:, off:off+w]
        # broadcasts neuron n's packed row across the 32 batch partitions
        def selcol(n):
            return sel8[:, n * B:(n + 1) * B]

        def b1row(n):
            return bvec[:, B1_OFF:B1_OFF + HID]

        def b2row(n):
            return bvec[:, B2_OFF:B2_OFF + HID]

        def b3row(n):
            return bvec[:, B3_OFF:B3_OFF + D]

        def gmrow(n):
            return bvec[:, GM_OFF:GM_OFF + D]

        def bmrow(n):
            return bvec[:, BM_OFF:BM_OFF + D]

        # ---- x setup: xT chunks [128, 32] f32r, k = 0..17 ----
        xT = []

        # proj part: projT = Wp.T @ emb.T + bp, chunks 0..1
        xe = cst.tile([B, D], f32, tag="xe")
        nc.sync.dma_start(out=xe[:], in_=emb)
        wpt = cst.tile([128, 2, D], f32r, tag="wpt")
        nc.gpsimd.dma_start(out=wpt[:], in_=wp.rearrange("(c p) d -> p c d", p=128))
        xeT = []
        for k in range(2):
            pt = trp.tile([128, 32], f32, tag="tr")
            nc.tensor.transpose(pt[:], xe[:, k * 128:(k + 1) * 128], eye[:])
            st = cst.tile([128, 32], f32r, tag=f"xeT{k}")
            nc.vector.tensor_copy(st[:], pt[:])
            xeT.append(st)
        for m in range(2):
            pp = trp.tile([128, 32], f32, tag="tr")
            for k in range(2):
                nc.tensor.matmul(pp[:], wpt[:, k, m * 128:(m + 1) * 128], xeT[k][:],
                                 start=(k == 0), stop=(k == 1))
            xt = xtp.tile([128, 32], f32r, tag="xt")
            nc.vector.tensor_scalar_add(xt[:], pp[:], bpt[:, m:m + 1])
            xT.append(xt)

        # hist part: chunks 2..17 broadcast across batch
        ht = cst.tile([16, 128], f32, tag="ht")
        nc.sync.dma_start(out=ht[:], in_=histd)
        pt = trp.tile([128, 16], f32, tag="tr")
        nc.tensor.transpose(pt[:], ht[:], eye[0:16, 0:16])
        histT = cst.tile([128, 16], f32, tag="histT")
        nc.vector.tensor_copy(histT[:], pt[:])
        for c in range(16):
            xt = xtp.tile([128, 32], f32r, tag="xt")
            nc.vector.tensor_scalar_mul(xt[:], onesb[:], histT[:, c:c + 1])
            xT.append(xt)

        # ---- main pipeline: emit_A(n) = GEMMs + gelus + centered y stats;
        # emit_B(n) = inv_std + modulated affine + output DMA.  B(n-1) is
        # emitted after A(n) so every engine keeps pipelined work and the
        # kernel tail is only B(last).
        ycs = {}
        stats = {}

        def dma_w1(n):
            w1t = []
            for s in range(4):
                t = w1p.tile([128, 4, HID], f32r, tag="w1")
                nc.gpsimd.dma_start(out=t[:], in_=w1d[n][:, 4 * s:4 * s + 4, :])
                w1t.append(t)
            t = w1p.tile([128, 4, HID], f32r, tag="w1")
            nc.gpsimd.dma_start(out=t[:, 0:2, :], in_=w1d[n][:, 16:18, :])
            w1t.append(t)
            return w1t

        def dma_w2(n):
            w2t = w23p.tile([128, KC2, HID], f32r, tag="w23")
            nc.gpsimd.dma_start(out=w2t[:], in_=w2d[n])
            return w2t

        def dma_w3(n):
            w3t = w23p.tile([128, KC2, D], f32r, tag="w23")
            nc.gpsimd.dma_start(out=w3t[:], in_=w3d[n])
            return w3t

        def transpose4(h):
            hT = []
            for j in range(KC2):
                pt = trp.tile([128, 32], f32, tag="tr")
                nc.tensor.transpose(pt[:], h[:, j * 128:(j + 1) * 128], eye[:])
                st = htp.tile([128, 32], f32r, tag="hT")
                nc.vector.tensor_copy(st[:], pt[:])
                hT.append(st)
            return hT

        def gemm1(n, w1t):
            p1 = accp.tile([B, HID], f32, tag="acc")
            nc.tensor.matmul(p1[:], selcol(n), b1row(n), start=True, stop=False)
            for k in range(KC1):
                nc.tensor.matmul(p1[:], xT[k][:], w1t[k // 4][:, k % 4, :],
                                 start=False, stop=(k == KC1 - 1))
            h1 = hp.tile([B, HID], f32, tag="h")
            nc.scalar.activation(h1[:], p1[:], GELU)
            return transpose4(h1)

        def gemm2(n, w2t, h1T):
            p2 = accp.tile([B, HID], f32, tag="acc")
            nc.tensor.matmul(p2[:], selcol(n), b2row(n), start=True, stop=False)
            for j in range(KC2):
                nc.tensor.matmul(p2[:], h1T[j][:], w2t[:, j, :],
                                 start=False, stop=(j == KC2 - 1))
            h2 = hp.tile([B, HID], f32, tag="h")
            nc.scalar.activation(h2[:], p2[:], GELU)
            return transpose4(h2)

        def gemm3(n, w3t, h2T):
            p3 = accp.tile([B, D], f32, tag="acc")
            nc.tensor.matmul(p3[:], selcol(n), b3row(n), start=True, stop=False)
            for j in range(KC2):
                nc.tensor.matmul(p3[:], h2T[j][:], w3t[:, j, :],
                                 start=False, stop=(j == KC2 - 1))

            # center y and accumulate sum(yc^2), all on DVE (no ACT table):
            #   rs = sum(y); yc = y - rs/D; ssq = sum(yc*yc)
            y = yp.tile([B, D], f32, tag="y")
            rs = rsp.tile([B, 1], f32, tag="rs")
            nc.scalar.activation(y[:], p3[:], COPY, accum_out=rs[:])
            nmu = stp.tile([B, 1], f32, tag="st")
            nc.vector.tensor_scalar_mul(nmu[:], rs[:], -1.0 / D)
            yc = ysp.tile([B, D], f32, tag="ys")
            nc.vector.tensor_scalar_add(yc[:], y[:], nmu[:])
            sqs = yp.tile([B, D], f32, tag="y")
            ssq = stp.tile([B, 1], f32, tag="st")
            nc.scalar.activation(sqs[:], yc[:], SQUARE, accum_out=ssq[:])
            ycs[n] = yc
            stats[n] = ssq

        def emit_A(n):
            # weights stream in consumption order: W1, W2, W3
            w1t = dma_w1(n)
            w2t = dma_w2(n)
            w3t = dma_w3(n)
            h1T = gemm1(n, w1t)
            h2T = gemm2(n, w2t, h1T)
            gemm3(n, w3t, h2T)

        def emit_B(n):
            yc, ssq = ycs[n], stats[n]
            std = stp.tile([B, 1], f32, tag="st")
            nc.scalar.activation(std[:], ssq[:], SQRT, bias=epst[:], scale=1.0 / D)
            inv = stp.tile([B, 1], f32, tag="st")
            nc.vector.reciprocal(inv[:], std[:])

            gb = gbp.tile([B, 2 * D], f32, tag="gb")
            nc.tensor.matmul(gb[:, 0:D], selcol(n), gmrow(n), start=True, stop=True)
            nc.tensor.matmul(gb[:, D:2 * D], selcol(n), bmrow(n), start=True, stop=True)

            yg = yp.tile([B, D], f32, tag="y")
            nc.vector.scalar_tensor_tensor(
                yg[:], yc[:], inv[:], gb[:, 0:D],
                mybir.AluOpType.mult, mybir.AluOpType.mult)
            yo = yp.tile([B, D], f32, tag="y")
            nc.vector.tensor_add(yo[:], yg[:], gb[:, D:2 * D])

            nc.sync.dma_start(out=out[:, n, :], in_=yo[:])

        # Neurons 0..NPC-3 run the standard pipeline with B lagging one
        # neuron.  The last two neurons interleave their DMA stream with the
        # layer pipeline so the final arriving bytes (W3 of the last neuron)
        # feed the shortest possible compute chain (GEMM3 + LN + output):
        # pool order ... W1[p] W2[p] W1[L] W3[p] W2[L] W3[L].
        p, L = NPC - 2, NPC - 1
        for n in range(p):
            emit_A(n)
            if n > 0:
                emit_B(n - 1)

        w1p_t = dma_w1(p)
        w2p_t = dma_w2(p)
        h1Tp = gemm1(p, w1p_t)
        h2Tp = gemm2(p, w2p_t, h1Tp)
        emit_B(p - 1)
        w1L_t = dma_w1(L)
        h1TL = gemm1(L, w1L_t)
        w2L_t = dma_w2(L)
        h2TL = gemm2(L, w2L_t, h1TL)
        w3p_t = dma_w3(p)
        gemm3(p, w3p_t, h2Tp)
        w3L_t = dma_w3(L)
        gemm3(L, w3L_t, h2TL)
        emit_B(p)
        emit_B(L)

    nc.compile()
    return nc


def _get_program():
    if "nc" not in _CACHE:
        _CACHE["nc"] = _build_program()
    return _CACHE["nc"]


def _prep_in_maps(input_embedding, pre_activations, Wp, bp, W1, b1, W2, b2, W3,
                  b3, gamma, beta, tick):
    emb = np.asarray(input_embedding, dtype=np.float32)
    hist = np.asarray(pre_activations, dtype=np.float32)
    Wp = np.asarray(Wp, dtype=np.float32)
    bp = np.asarray(bp, dtype=np.float32)
    W1 = np.asarray(W1, dtype=np.float32)
    b1 = np.asarray(b1, dtype=np.float32)
    W2 = np.asarray(W2, dtype=np.float32)
    b2 = np.asarray(b2, dtype=np.float32)
    W3 = np.asarray(W3, dtype=np.float32)
    b3 = np.asarray(b3, dtype=np.float32)
    gamma = np.asarray(gamma, dtype=np.float32)
    beta = np.asarray(beta, dtype=np.float32)

    # oscillator modulation folded into gamma/beta
    i = np.arange(N_NEURONS, dtype=np.float64)
    freq = FMIN * (FMAX / FMIN) ** (i / (N_NEURONS - 1))
    phase = np.mod(i * 2.3571, 2.0 * math.pi)
    t = float(np.asarray(tick)) * TICK_INTERVAL
    mod = (1.0 + 0.5 * np.sin(2.0 * math.pi * freq * t + phase)).astype(np.float32)
    gm = (gamma * mod[:, None]).astype(np.float32)
    bm = (beta * mod[:, None]).astype(np.float32)

    histd = np.ascontiguousarray(hist.reshape(16, 128))
    bpd = np.ascontiguousarray(bp.reshape(2, 128).T)
    eyed = np.eye(32, dtype=np.float32)

    # weight layout: (n, p, k_chunk, hid) so each supertile DMA reads one
    # contiguous run per partition
    W1r = np.ascontiguousarray(
        W1.reshape(N_NEURONS, KC1, 128, HID).transpose(0, 2, 1, 3))
    W2r = np.ascontiguousarray(
        W2.reshape(N_NEURONS, KC2, 128, HID).transpose(0, 2, 1, 3))
    W3r = np.ascontiguousarray(
        W3.reshape(N_NEURONS, KC2, 128, D).transpose(0, 2, 1, 3))

    # one-hot selector: sel8[k, n*B + j] = (k == n), broadcasts bvec row n
    # across the batch partitions via a K=8 matmul
    sel8 = np.zeros((NPC, NPC * B), dtype=np.float32)
    for n in range(NPC):
        sel8[n, n * B:(n + 1) * B] = 1.0

    in_maps = []
    for c in range(N_CORES):
        s = slice(c * NPC, (c + 1) * NPC)
        bvec = np.concatenate([b1[s], b2[s], b3[s], gm[s], bm[s]], axis=1)
        in_maps.append({
            "emb": emb,
            "wp": Wp,
            "bpd": bpd,
            "histd": histd,
            "eyed": eyed,
            "w1d": W1r[s],
            "w2d": W2r[s],
            "w3d": W3r[s],
            "bvecd": np.ascontiguousarray(bvec),
            "sel8d": sel8,
        })
    return in_maps


def run(inputs, trace=False):
    nc = _get_program()
    in_maps = _prep_in_maps(**inputs)
    br = run_bass_kernel_spmd(nc, in_maps, core_ids=list(range(N_CORES)),
                              trace=trace)
    out = np.concatenate([r["out"] for r in br.results], axis=1)
    return np.ascontiguousarray(out, dtype=np.float32), br


def kernel(**inputs) -> np.ndarray:
    out, _ = run(inputs, trace=False)
    return out



# revision 11
# speedup vs baseline: 1.5705x; 1.5705x over previous
"""NeuronPool (moe_routing) Trainium2 kernel.

Expert-parallel over 8 NeuronCores: core c computes neurons [8c, 8c+8) for the
full batch, host concatenates along the neuron axis.

Per-core pipeline (all shapes per core):
  x = [proj | hist_broadcast]  (built on device, stored transposed as 18
      [128,32] f32r tiles so the batch stays on the PSUM partition dim)
  A(n), per neuron:
      psum1[32,512] = sel(n).T @ b1_rows  +  sum_k xT[k].T @ W1[n,k]   (f32r;
          biases/gamma/beta live one-neuron-per-partition and broadcast via a
          K=8 one-hot selector matmul)
      h1 = gelu(psum1)                 -> PE-transpose -> h1T [128,32] x4
      psum2[32,512] = bias + sum_j h1T[j].T @ W2[n,j]
      h2 = gelu(psum2)                 -> PE-transpose -> h2T
      psum3[32,256] = bias + sum_j h2T[j].T @ W3[n,j]
      y = copy(psum3) + row sums (ACT accum_out); yc = y - mean; ssq(yc)
  B(n), emitted one neuron behind A so it pipelines instead of trailing:
      inv_std = 1/sqrt(ssq/D + eps); out = yc*inv_std*(gamma*mod) + beta*mod
The last two neurons' weight DMAs interleave with the layer pipeline so the
final arriving bytes (W3 of the last neuron) feed the shortest compute chain.
Weights are cast to bf16 on the host and stream HBM->SBUF at 2 bytes/elem
(halves the HBM-bound weight traffic vs fp32; activations pair in bf16 so
GEMMs run at 1 cycle/row).  Measured rel err 3.9e-3 in numpy sim vs the
2e-2 gate.
"""
import math
import numpy as np
import ml_dtypes
from contextlib import ExitStack

import concourse.bass as bass
import concourse.tile as tile
from concourse import bacc, mybir
from concourse.bass_utils import run_bass_kernel_spmd

N_CORES = 8
B = 32          # batch
D = 256         # model dim
HIST = 8
HID = 512
N_NEURONS = 64
NPC = N_NEURONS // N_CORES  # 8 neurons per core
IN_DIM = D * (1 + HIST)     # 2304
KC1 = IN_DIM // 128         # 18 contraction chunks for GEMM1
KC2 = HID // 128            # 4 chunks for GEMM2/GEMM3
LN_EPS = 1e-5
FMIN, FMAX = 0.5, 40.0
TICK_INTERVAL = 0.1

f32 = mybir.dt.float32
f32r = mybir.dt.float32r
bf16 = mybir.dt.bfloat16

# packed per-neuron row layout (columns in bvec8: one SBUF partition per
# neuron, broadcast into PSUM via a K=8 one-hot selector matmul)
B1_OFF = 0
B2_OFF = B1_OFF + HID
B3_OFF = B2_OFF + HID
GM_OFF = B3_OFF + D
BM_OFF = GM_OFF + D
BVEC_LEN = BM_OFF + D

_CACHE = {}


def _build_program():
    nc = bacc.Bacc("TRN2", target_bir_lowering=False, debug=False,
                   num_devices=N_CORES)

    emb = nc.dram_tensor("emb", [B, D], f32, kind="ExternalInput").ap()
    wp = nc.dram_tensor("wp", [D, D], f32, kind="ExternalInput").ap()
    bpd = nc.dram_tensor("bpd", [128, 2], f32, kind="ExternalInput").ap()
    histd = nc.dram_tensor("histd", [16, 128], f32, kind="ExternalInput").ap()
    eyed = nc.dram_tensor("eyed", [32, 32], f32, kind="ExternalInput").ap()
    w1d = nc.dram_tensor("w1d", [NPC, 128, KC1, HID], bf16, kind="ExternalInput").ap()
    w2d = nc.dram_tensor("w2d", [NPC, 128, KC2, HID], bf16, kind="ExternalInput").ap()
    w3d = nc.dram_tensor("w3d", [NPC, 128, KC2, D], bf16, kind="ExternalInput").ap()
    bvecd = nc.dram_tensor("bvecd", [NPC, BVEC_LEN], f32, kind="ExternalInput").ap()
    sel8d = nc.dram_tensor("sel8d", [NPC, NPC * B], f32, kind="ExternalInput").ap()
    out = nc.dram_tensor("out", [B, NPC, D], f32, kind="ExternalOutput").ap()

    GELU = mybir.ActivationFunctionType.Gelu
    COPY = mybir.ActivationFunctionType.Copy
    SQUARE = mybir.ActivationFunctionType.Square
    SQRT = mybir.ActivationFunctionType.Sqrt

    with tile.TileContext(nc) as tc, ExitStack() as ctx:
        # SBUF pools
        cst = ctx.enter_context(tc.tile_pool(name="cst", bufs=1))
        xtp = ctx.enter_context(tc.tile_pool(name="xtp", bufs=KC1))
        w1p = ctx.enter_context(tc.tile_pool(name="w1p", bufs=9))
        w23p = ctx.enter_context(tc.tile_pool(name="w23p", bufs=6))
        htp = ctx.enter_context(tc.tile_pool(name="htp", bufs=16))
        hp = ctx.enter_context(tc.tile_pool(name="hp", bufs=4))
        ysp = ctx.enter_context(tc.tile_pool(name="ysp", bufs=NPC))
        rsp = ctx.enter_context(tc.tile_pool(name="rsp", bufs=NPC))
        yp = ctx.enter_context(tc.tile_pool(name="yp", bufs=10))
        stp = ctx.enter_context(tc.tile_pool(name="stp", bufs=12))
        # PSUM pools (8 banks total: 3 + 3 + 2)
        accp = ctx.enter_context(tc.tile_pool(name="accp", bufs=3, space="PSUM"))
        trp = ctx.enter_context(tc.tile_pool(name="trp", bufs=3, space="PSUM"))
        gbp = ctx.enter_context(tc.tile_pool(name="gbp", bufs=2, space="PSUM"))

        # ---- constants ----
        eye = cst.tile([32, 32], f32, tag="eye")
        nc.sync.dma_start(out=eye[:], in_=eyed)
        onesf = cst.tile([1, 32], f32, tag="onesf")
        nc.vector.memset(onesf[:], 1.0)
        onesr = cst.tile([1, 32], f32r, tag="onesr")
        nc.vector.tensor_copy(onesr[:], onesf[:])
        onesb = cst.tile([128, 32], f32, tag="onesb")
        nc.vector.memset(onesb[:], 1.0)
        epst = cst.tile([B, 1], f32, tag="epst")
        nc.vector.memset(epst[:], LN_EPS)
        bpt = cst.tile([128, 2], f32, tag="bpt")
        nc.sync.dma_start(out=bpt[:], in_=bpd)
        bvec = cst.tile([NPC, BVEC_LEN], f32r, tag="bvec")
        nc.gpsimd.dma_start(out=bvec[:], in_=bvecd)
        sel8 = cst.tile([NPC, NPC * B], f32r, tag="sel8")
        nc.gpsimd.dma_start(out=sel8[:], in_=sel8d)

        # K=8 one-hot selector: sel8[:, 32n:32n+32].T @ bvec[:, off:off+w]
        # broadcasts neuron n's packed row across the 32 batch partitions
        def selcol(n):
            return sel8[:, n * B:(n + 1) * B]

        def b1row(n):
            return bvec[:, B1_OFF:B1_OFF + HID]

        def b2row(n):
            return bvec[:, B2_OFF:B2_OFF + HID]

        def b3row(n):
            return bvec[:, B3_OFF:B3_OFF + D]

        def gmrow(n):
            return bvec[:, GM_OFF:GM_OFF + D]

        def bmrow(n):
            return bvec[:, BM_OFF:BM_OFF + D]

        # ---- x setup: xT chunks [128, 32] f32r, k = 0..17 ----
        xT = []

        # proj part: projT = Wp.T @ emb.T + bp, chunks 0..1
        xe = cst.tile([B, D], f32, tag="xe")
        nc.sync.dma_start(out=xe[:], in_=emb)
        wpt = cst.tile([128, 2, D], f32r, tag="wpt")
        nc.gpsimd.dma_start(out=wpt[:], in_=wp.rearrange("(c p) d -> p c d", p=128))
        xeT = []
        for k in range(2):
            pt = trp.tile([128, 32], f32, tag="tr")
            nc.tensor.transpose(pt[:], xe[:, k * 128:(k + 1) * 128], eye[:])
            st = cst.tile([128, 32], f32r, tag=f"xeT{k}")
            nc.vector.tensor_copy(st[:], pt[:])
            xeT.append(st)
        for m in range(2):
            pp = trp.tile([128, 32], f32, tag="tr")
            for k in range(2):
                nc.tensor.matmul(pp[:], wpt[:, k, m * 128:(m + 1) * 128], xeT[k][:],
                                 start=(k == 0), stop=(k == 1))
            xt = xtp.tile([128, 32], bf16, tag="xt")
            nc.vector.tensor_scalar_add(xt[:], pp[:], bpt[:, m:m + 1])
            xT.append(xt)

        # hist part: chunks 2..17 broadcast across batch
        ht = cst.tile([16, 128], f32, tag="ht")
        nc.sync.dma_start(out=ht[:], in_=histd)
        pt = trp.tile([128, 16], f32, tag="tr")
        nc.tensor.transpose(pt[:], ht[:], eye[0:16, 0:16])
        histT = cst.tile([128, 16], f32, tag="histT")
        nc.vector.tensor_copy(histT[:], pt[:])
        for c in range(16):
            xt = xtp.tile([128, 32], bf16, tag="xt")
            nc.vector.tensor_scalar_mul(xt[:], onesb[:], histT[:, c:c + 1])
            xT.append(xt)

        # ---- main pipeline: emit_A(n) = GEMMs + gelus + centered y stats;
        # emit_B(n) = inv_std + modulated affine + output DMA.  B(n-1) is
        # emitted after A(n) so every engine keeps pipelined work and the
        # kernel tail is only B(last).
        ycs = {}
        stats = {}

        def dma_w1(n):
            w1t = []
            for s in range(3):
                t = w1p.tile([128, 6, HID], bf16, tag="w1")
                nc.gpsimd.dma_start(out=t[:], in_=w1d[n][:, 6 * s:6 * s + 6, :])
                w1t.append(t)
            return w1t

        def dma_w2(n):
            w2t = w23p.tile([128, KC2, HID], bf16, tag="w23")
            nc.gpsimd.dma_start(out=w2t[:], in_=w2d[n])
            return w2t

        def dma_w3(n):
            w3t = w23p.tile([128, KC2, D], bf16, tag="w23")
            nc.gpsimd.dma_start(out=w3t[:], in_=w3d[n])
            return w3t

        def transpose4(h):
            hT = []
            for j in range(KC2):
                pt = trp.tile([128, 32], f32, tag="tr")
                nc.tensor.transpose(pt[:], h[:, j * 128:(j + 1) * 128], eye[:])
                st = htp.tile([128, 32], bf16, tag="hT")
                nc.vector.tensor_copy(st[:], pt[:])
                hT.append(st)
            return hT

        def gemm1(n, w1t):
            p1 = accp.tile([B, HID], f32, tag="acc")
            nc.tensor.matmul(p1[:], selcol(n), b1row(n), start=True, stop=False)
            for k in range(KC1):
                nc.tensor.matmul(p1[:], xT[k][:], w1t[k // 6][:, k % 6, :],
                                 start=False, stop=(k == KC1 - 1))
            h1 = hp.tile([B, HID], f32, tag="h")
            nc.scalar.activation(h1[:], p1[:], GELU)
            return transpose4(h1)

        def gemm2(n, w2t, h1T):
            p2 = accp.tile([B, HID], f32, tag="acc")
            nc.tensor.matmul(p2[:], selcol(n), b2row(n), start=True, stop=False)
            for j in range(KC2):
                nc.tensor.matmul(p2[:], h1T[j][:], w2t[:, j, :],
                                 start=False, stop=(j == KC2 - 1))
            h2 = hp.tile([B, HID], f32, tag="h")
            nc.scalar.activation(h2[:], p2[:], GELU)
            return transpose4(h2)

        def gemm3(n, w3t, h2T):
            p3 = accp.tile([B, D], f32, tag="acc")
            nc.tensor.matmul(p3[:], selcol(n), b3row(n), start=True, stop=False)
            for j in range(KC2):
                nc.tensor.matmul(p3[:], h2T[j][:], w3t[:, j, :],
                                 start=False, stop=(j == KC2 - 1))

            # center y and accumulate sum(yc^2), all on DVE (no ACT table):
            #   rs = sum(y); yc = y - rs/D; ssq = sum(yc*yc)
            y = yp.tile([B, D], f32, tag="y")
            rs = rsp.tile([B, 1], f32, tag="rs")
            nc.scalar.activation(y[:], p3[:], COPY, accum_out=rs[:])
            nmu = stp.tile([B, 1], f32, tag="st")
            nc.vector.tensor_scalar_mul(nmu[:], rs[:], -1.0 / D)
            yc = ysp.tile([B, D], f32, tag="ys")
            nc.vector.tensor_scalar_add(yc[:], y[:], nmu[:])
            sqs = yp.tile([B, D], f32, tag="y")
            ssq = stp.tile([B, 1], f32, tag="st")
            nc.scalar.activation(sqs[:], yc[:], SQUARE, accum_out=ssq[:])
            ycs[n] = yc
            stats[n] = ssq

        def emit_A(n):
            # weights stream in consumption order: W1, W2, W3
            w1t = dma_w1(n)
            w2t = dma_w2(n)
            w3t = dma_w3(n)
            h1T = gemm1(n, w1t)
            h2T = gemm2(n, w2t, h1T)
            gemm3(n, w3t, h2T)

        def emit_B(n):
            yc, ssq = ycs[n], stats[n]
            std = stp.tile([B, 1], f32, tag="st")
            nc.scalar.activation(std[:], ssq[:], SQRT, bias=epst[:], scale=1.0 / D)
            inv = stp.tile([B, 1], f32, tag="st")
            nc.vector.reciprocal(inv[:], std[:])

            gb = gbp.tile([B, 2 * D], f32, tag="gb")
            nc.tensor.matmul(gb[:], selcol(n), bvec[:, GM_OFF:GM_OFF + 2 * D],
                             start=True, stop=True)

            yg = yp.tile([B, D], f32, tag="y")
            nc.vector.scalar_tensor_tensor(
                yg[:], yc[:], inv[:], gb[:, 0:D],
                mybir.AluOpType.mult, mybir.AluOpType.mult)
            yo = yp.tile([B, D], f32, tag="y")
            nc.vector.tensor_add(yo[:], yg[:], gb[:, D:2 * D])

            nc.sync.dma_start(out=out[:, n, :], in_=yo[:])

        # Neurons 0..NPC-3 run the standard pipeline with B lagging one
        # neuron.  The last two neurons interleave their DMA stream with the
        # layer pipeline so the final arriving bytes (W3 of the last neuron)
        # feed the shortest possible compute chain (GEMM3 + LN + output):
        # pool order ... W1[p] W2[p] W1[L] W3[p] W2[L] W3[L].
        p, L = NPC - 2, NPC - 1
        for n in range(p):
            emit_A(n)
            if n > 0:
                emit_B(n - 1)

        w1p_t = dma_w1(p)
        w2p_t = dma_w2(p)
        h1Tp = gemm1(p, w1p_t)
        h2Tp = gemm2(p, w2p_t, h1Tp)
        emit_B(p - 1)
        w1L_t = dma_w1(L)
        h1TL = gemm1(L, w1L_t)
        w2L_t = dma_w2(L)
        h2TL = gemm2(L, w2L_t, h1TL)
        w3p_t = dma_w3(p)
        gemm3(p, w3p_t, h2Tp)
        w3L_t = dma_w3(L)
        gemm3(L, w3L_t, h2TL)
        emit_B(p)
        emit_B(L)

    nc.compile()
    return nc


def _get_program():
    if "nc" not in _CACHE:
        _CACHE["nc"] = _build_program()
    return _CACHE["nc"]


def _prep_in_maps(input_embedding, pre_activations, Wp, bp, W1, b1, W2, b2, W3,
                  b3, gamma, beta, tick):
    emb = np.asarray(input_embedding, dtype=np.float32)
    hist = np.asarray(pre_activations, dtype=np.float32)
    Wp = np.asarray(Wp, dtype=np.float32)
    bp = np.asarray(bp, dtype=np.float32)
    W1 = np.asarray(W1, dtype=np.float32)
    b1 = np.asarray(b1, dtype=np.float32)
    W2 = np.asarray(W2, dtype=np.float32)
    b2 = np.asarray(b2, dtype=np.float32)
    W3 = np.asarray(W3, dtype=np.float32)
    b3 = np.asarray(b3, dtype=np.float32)
    gamma = np.asarray(gamma, dtype=np.float32)
    beta = np.asarray(beta, dtype=np.float32)

    # oscillator modulation folded into gamma/beta
    i = np.arange(N_NEURONS, dtype=np.float64)
    freq = FMIN * (FMAX / FMIN) ** (i / (N_NEURONS - 1))
    phase = np.mod(i * 2.3571, 2.0 * math.pi)
    t = float(np.asarray(tick)) * TICK_INTERVAL
    mod = (1.0 + 0.5 * np.sin(2.0 * math.pi * freq * t + phase)).astype(np.float32)
    gm = (gamma * mod[:, None]).astype(np.float32)
    bm = (beta * mod[:, None]).astype(np.float32)

    histd = np.ascontiguousarray(hist.reshape(16, 128))
    bpd = np.ascontiguousarray(bp.reshape(2, 128).T)
    eyed = np.eye(32, dtype=np.float32)

    # weight layout: (n, p, k_chunk, hid) so each supertile DMA reads one
    # contiguous run per partition; bf16 in HBM halves the DMA traffic
    W1r = np.ascontiguousarray(
        W1.reshape(N_NEURONS, KC1, 128, HID).transpose(0, 2, 1, 3)).astype(
            ml_dtypes.bfloat16)
    W2r = np.ascontiguousarray(
        W2.reshape(N_NEURONS, KC2, 128, HID).transpose(0, 2, 1, 3)).astype(
            ml_dtypes.bfloat16)
    W3r = np.ascontiguousarray(
        W3.reshape(N_NEURONS, KC2, 128, D).transpose(0, 2, 1, 3)).astype(
            ml_dtypes.bfloat16)

    # one-hot selector: sel8[k, n*B + j] = (k == n), broadcasts bvec row n
    # across the batch partitions via a K=8 matmul
    sel8 = np.zeros((NPC, NPC * B), dtype=np.float32)
    for n in range(NPC):
        sel8[n, n * B:(n + 1) * B] = 1.0

    in_maps = []
    for c in range(N_CORES):
        s = slice(c * NPC, (c + 1) * NPC)
        bvec = np.concatenate([b1[s], b2[s], b3[s], gm[s], bm[s]], axis=1)
        in_maps.append({
            "emb": emb,
            "wp": Wp,
            "bpd": bpd,
            "histd": histd,
            "eyed": eyed,
            "w1d": W1r[s],
            "w2d": W2r[s],
            "w3d": W3r[s],
            "bvecd": np.ascontiguousarray(bvec),
            "sel8d": sel8,
        })
    return in_maps


def run(inputs, trace=False):
    nc = _get_program()
    in_maps = _prep_in_maps(**inputs)
    br = run_bass_kernel_spmd(nc, in_maps, core_ids=list(range(N_CORES)),
                              trace=trace)
    out = np.concatenate([r["out"] for r in br.results], axis=1)
    return np.ascontiguousarray(out, dtype=np.float32), br


def kernel(**inputs) -> np.ndarray:
    out, _ = run(inputs, trace=False)
    return out



# revision 16
# speedup vs baseline: 1.7417x; 1.1090x over previous
"""NeuronPool (moe_routing) Trainium2 kernel.

Expert-parallel over 8 NeuronCores: core c computes neurons [8c, 8c+8) for the
full batch, host concatenates along the neuron axis.

Per-core pipeline (all shapes per core):
  x = [proj | hist_broadcast]  (built on device, stored transposed as 18
      [128,32] f32r tiles so the batch stays on the PSUM partition dim)
  A(n), per neuron:
      psum1[32,512] = sel(n).T @ b1_rows  +  sum_k xT[k].T @ W1[n,k]   (f32r;
          biases/gamma/beta live one-neuron-per-partition and broadcast via a
          K=8 one-hot selector matmul)
      h1 = gelu(psum1)                 -> PE-transpose -> h1T [128,32] x4
      psum2[32,512] = bias + sum_j h1T[j].T @ W2[n,j]
      h2 = gelu(psum2)                 -> PE-transpose -> h2T
      psum3[32,256] = bias + sum_j h2T[j].T @ W3[n,j]
      y = copy(psum3) + row sums (ACT accum_out); yc = y - mean; ssq(yc)
  B(n), emitted one neuron behind A so it pipelines instead of trailing:
      inv_std = 1/sqrt(ssq/D + eps); out = yc*inv_std*(gamma*mod) + beta*mod
The last two neurons' weight DMAs interleave with the layer pipeline so the
final arriving bytes (W3 of the last neuron) feed the shortest compute chain.
Weights are cast to bf16 on the host and stream HBM->SBUF at 2 bytes/elem
(halves the HBM-bound weight traffic vs fp32; activations pair in bf16 so
GEMMs run at 1 cycle/row).  Measured rel err 3.9e-3 in numpy sim vs the
2e-2 gate.
"""
import math
import numpy as np
import ml_dtypes
from contextlib import ExitStack

import concourse.bass as bass
import concourse.tile as tile
from concourse import bacc, mybir
from concourse.bass_utils import run_bass_kernel_spmd

N_CORES = 8
B = 32          # batch
D = 256         # model dim
HIST = 8
HID = 512
N_NEURONS = 64
NPC = N_NEURONS // N_CORES  # 8 neurons per core
IN_DIM = D * (1 + HIST)     # 2304
KC1 = IN_DIM // 128         # 18 contraction chunks for GEMM1
KC2 = HID // 128            # 4 chunks for GEMM2/GEMM3
LN_EPS = 1e-5
FMIN, FMAX = 0.5, 40.0
TICK_INTERVAL = 0.1

f32 = mybir.dt.float32
f32r = mybir.dt.float32r
bf16 = mybir.dt.bfloat16

# packed per-neuron row layout (columns in bvec8: one SBUF partition per
# neuron, broadcast into PSUM via a K=8 one-hot selector matmul)
B1_OFF = 0
B2_OFF = B1_OFF + HID
B3_OFF = B2_OFF + HID
GM_OFF = B3_OFF + D
BM_OFF = GM_OFF + D
BVEC_LEN = BM_OFF + D

_CACHE = {}


def _build_program():
    nc = bacc.Bacc("TRN2", target_bir_lowering=False, debug=False,
                   num_devices=N_CORES)

    emb = nc.dram_tensor("emb", [B, D], f32, kind="ExternalInput").ap()
    wp = nc.dram_tensor("wp", [D, D], f32, kind="ExternalInput").ap()
    bpd = nc.dram_tensor("bpd", [128, 2], f32, kind="ExternalInput").ap()
    histd = nc.dram_tensor("histd", [16, 128], f32, kind="ExternalInput").ap()
    eyed = nc.dram_tensor("eyed", [32, 32], f32, kind="ExternalInput").ap()
    w1d = nc.dram_tensor("w1d", [NPC, 128, KC1, HID], bf16, kind="ExternalInput").ap()
    w2d = nc.dram_tensor("w2d", [NPC, 128, KC2, HID], bf16, kind="ExternalInput").ap()
    w3d = nc.dram_tensor("w3d", [NPC, 128, KC2, D], bf16, kind="ExternalInput").ap()
    bvecd = nc.dram_tensor("bvecd", [NPC, BVEC_LEN], f32, kind="ExternalInput").ap()
    sel8d = nc.dram_tensor("sel8d", [NPC, NPC * B], f32, kind="ExternalInput").ap()
    out = nc.dram_tensor("out", [B, NPC, D], f32, kind="ExternalOutput").ap()

    GELU = mybir.ActivationFunctionType.Gelu
    COPY = mybir.ActivationFunctionType.Copy
    SQUARE = mybir.ActivationFunctionType.Square
    SQRT = mybir.ActivationFunctionType.Sqrt

    with tile.TileContext(nc) as tc, ExitStack() as ctx:
        # SBUF pools
        cst = ctx.enter_context(tc.tile_pool(name="cst", bufs=1))
        xtp = ctx.enter_context(tc.tile_pool(name="xtp", bufs=KC1))
        w1p = ctx.enter_context(tc.tile_pool(name="w1p", bufs=8))
        w23p = ctx.enter_context(tc.tile_pool(name="w23p", bufs=8))
        htp = ctx.enter_context(tc.tile_pool(name="htp", bufs=16))
        hp = ctx.enter_context(tc.tile_pool(name="hp", bufs=4))
        ysp = ctx.enter_context(tc.tile_pool(name="ysp", bufs=NPC))
        rsp = ctx.enter_context(tc.tile_pool(name="rsp", bufs=NPC))
        yp = ctx.enter_context(tc.tile_pool(name="yp", bufs=10))
        stp = ctx.enter_context(tc.tile_pool(name="stp", bufs=12))
        # PSUM pools (8 banks total: 3 + 3 + 2)
        accp = ctx.enter_context(tc.tile_pool(name="accp", bufs=3, space="PSUM"))
        trp = ctx.enter_context(tc.tile_pool(name="trp", bufs=3, space="PSUM"))
        gbp = ctx.enter_context(tc.tile_pool(name="gbp", bufs=2, space="PSUM"))

        # ---- constants ----
        eye = cst.tile([32, 32], f32, tag="eye")
        nc.sync.dma_start(out=eye[:], in_=eyed)
        onesf = cst.tile([1, 32], f32, tag="onesf")
        nc.vector.memset(onesf[:], 1.0)
        onesr = cst.tile([1, 32], f32r, tag="onesr")
        nc.vector.tensor_copy(onesr[:], onesf[:])
        onesb = cst.tile([128, 32], f32, tag="onesb")
        nc.vector.memset(onesb[:], 1.0)
        epst = cst.tile([B, 1], f32, tag="epst")
        nc.vector.memset(epst[:], LN_EPS)
        bpt = cst.tile([128, 2], f32, tag="bpt")
        nc.sync.dma_start(out=bpt[:], in_=bpd)
        bvec = cst.tile([NPC, BVEC_LEN], f32r, tag="bvec")
        nc.gpsimd.dma_start(out=bvec[:], in_=bvecd)
        sel8 = cst.tile([NPC, NPC * B], f32r, tag="sel8")
        nc.gpsimd.dma_start(out=sel8[:], in_=sel8d)

        # K=8 one-hot selector: sel8[:, 32n:32n+32].T @ bvec[:, off:off+w]
        # broadcasts neuron n's packed row across the 32 batch partitions
        def selcol(n):
            return sel8[:, n * B:(n + 1) * B]

        def b1row(n):
            return bvec[:, B1_OFF:B1_OFF + HID]

        def b2row(n):
            return bvec[:, B2_OFF:B2_OFF + HID]

        def b3row(n):
            return bvec[:, B3_OFF:B3_OFF + D]

        def gmrow(n):
            return bvec[:, GM_OFF:GM_OFF + D]

        def bmrow(n):
            return bvec[:, BM_OFF:BM_OFF + D]

        # ---- x setup: xT chunks [128, 32] f32r, k = 0..17 ----
        xT = []

        # proj part: projT = Wp.T @ emb.T + bp, chunks 0..1
        xe = cst.tile([B, D], f32, tag="xe")
        nc.sync.dma_start(out=xe[:], in_=emb)
        wpt = cst.tile([128, 2, D], f32r, tag="wpt")
        nc.gpsimd.dma_start(out=wpt[:], in_=wp.rearrange("(c p) d -> p c d", p=128))
        xeT = []
        for k in range(2):
            pt = trp.tile([128, 32], f32, tag="tr")
            nc.tensor.transpose(pt[:], xe[:, k * 128:(k + 1) * 128], eye[:])
            st = cst.tile([128, 32], f32r, tag=f"xeT{k}")
            nc.vector.tensor_copy(st[:], pt[:])
            xeT.append(st)
        for m in range(2):
            pp = trp.tile([128, 32], f32, tag="tr")
            for k in range(2):
                nc.tensor.matmul(pp[:], wpt[:, k, m * 128:(m + 1) * 128], xeT[k][:],
                                 start=(k == 0), stop=(k == 1))
            xt = xtp.tile([128, 32], bf16, tag="xt")
            nc.vector.tensor_scalar_add(xt[:], pp[:], bpt[:, m:m + 1])
            xT.append(xt)

        # hist part: chunks 2..17 broadcast across batch
        ht = cst.tile([16, 128], f32, tag="ht")
        nc.sync.dma_start(out=ht[:], in_=histd)
        pt = trp.tile([128, 16], f32, tag="tr")
        nc.tensor.transpose(pt[:], ht[:], eye[0:16, 0:16])
        histT = cst.tile([128, 16], f32, tag="histT")
        nc.vector.tensor_copy(histT[:], pt[:])
        for c in range(16):
            xt = xtp.tile([128, 32], bf16, tag="xt")
            nc.vector.tensor_scalar_mul(xt[:], onesb[:], histT[:, c:c + 1])
            xT.append(xt)

        # ---- main pipeline: emit_A(n) = GEMMs + gelus + centered y stats;
        # emit_B(n) = inv_std + modulated affine + output DMA.  B(n-1) is
        # emitted after A(n) so every engine keeps pipelined work and the
        # kernel tail is only B(last).
        ycs = {}
        stats = {}

        def dma_w1(n):
            w1t = []
            for s in range(2):
                t = w1p.tile([128, 9, HID], bf16, tag="w1")
                nc.gpsimd.dma_start(out=t[:], in_=w1d[n][:, 9 * s:9 * s + 9, :])
                w1t.append(t)
            return w1t

        def dma_w2(n):
            w2t = w23p.tile([128, KC2, HID], bf16, tag="w23")
            nc.gpsimd.dma_start(out=w2t[:], in_=w2d[n])
            return w2t

        def dma_w3(n):
            w3t = w23p.tile([128, KC2, D], bf16, tag="w23")
            nc.gpsimd.dma_start(out=w3t[:], in_=w3d[n])
            return w3t

        def transpose4(h):
            hT = []
            for j in range(KC2):
                pt = trp.tile([128, 32], f32, tag="tr")
                nc.tensor.transpose(pt[:], h[:, j * 128:(j + 1) * 128], eye[:])
                st = htp.tile([128, 32], bf16, tag="hT")
                nc.vector.tensor_copy(st[:], pt[:])
                hT.append(st)
            return hT

        def gemm1(n, w1t):
            # matmuls + gelu only; the dependent transposes are emitted later
            # (emit_rest) so the PE queue never stalls on the ACT gelu
            p1 = accp.tile([B, HID], f32, tag="acc")
            nc.tensor.matmul(p1[:], selcol(n), b1row(n), start=True, stop=False)
            for k in range(KC1):
                nc.tensor.matmul(p1[:], xT[k][:], w1t[k // 9][:, k % 9, :],
                                 start=False, stop=(k == KC1 - 1))
            h1 = hp.tile([B, HID], f32, tag="h")
            nc.scalar.activation(h1[:], p1[:], GELU)
            return h1

        def gemm2(n, w2t, h1T):
            p2 = accp.tile([B, HID], f32, tag="acc")
            nc.tensor.matmul(p2[:], selcol(n), b2row(n), start=True, stop=False)
            for j in range(KC2):
                nc.tensor.matmul(p2[:], h1T[j][:], w2t[:, j, :],
                                 start=False, stop=(j == KC2 - 1))
            h2 = hp.tile([B, HID], f32, tag="h")
            nc.scalar.activation(h2[:], p2[:], GELU)
            return h2

        def gemm3(n, w3t, h2T):
            p3 = accp.tile([B, D], f32, tag="acc")
            nc.tensor.matmul(p3[:], selcol(n), b3row(n), start=True, stop=False)
            for j in range(KC2):
                nc.tensor.matmul(p3[:], h2T[j][:], w3t[:, j, :],
                                 start=False, stop=(j == KC2 - 1))

            # center y and accumulate sum(yc^2), all on DVE (no ACT table):
            #   rs = sum(y); yc = y - rs/D; ssq = sum(yc*yc)
            y = yp.tile([B, D], f32, tag="y")
            rs = rsp.tile([B, 1], f32, tag="rs")
            nc.scalar.activation(y[:], p3[:], COPY, accum_out=rs[:])
            nmu = stp.tile([B, 1], f32, tag="st")
            nc.vector.tensor_scalar_mul(nmu[:], rs[:], -1.0 / D)
            yc = ysp.tile([B, D], f32, tag="ys")
            nc.vector.tensor_scalar_add(yc[:], y[:], nmu[:])
            sqs = yp.tile([B, D], f32, tag="y")
            ssq = stp.tile([B, 1], f32, tag="st")
            nc.scalar.activation(sqs[:], yc[:], SQUARE, accum_out=ssq[:])
            ycs[n] = yc
            stats[n] = ssq

        def emit_B(n):
            yc, ssq = ycs[n], stats[n]
            std = stp.tile([B, 1], f32, tag="st")
            nc.scalar.activation(std[:], ssq[:], SQRT, bias=epst[:], scale=1.0 / D)
            inv = stp.tile([B, 1], f32, tag="st")
            nc.vector.reciprocal(inv[:], std[:])

            gb = gbp.tile([B, 2 * D], f32, tag="gb")
            nc.tensor.matmul(gb[:], selcol(n), bvec[:, GM_OFF:GM_OFF + 2 * D],
                             start=True, stop=True)

            yg = yp.tile([B, D], f32, tag="y")
            nc.vector.scalar_tensor_tensor(
                yg[:], yc[:], inv[:], gb[:, 0:D],
                mybir.AluOpType.mult, mybir.AluOpType.mult)
            yo = yp.tile([B, D], f32, tag="y")
            nc.vector.tensor_add(yo[:], yg[:], gb[:, D:2 * D])

            nc.sync.dma_start(out=out[:, n, :], in_=yo[:])

        # Software pipeline, one neuron deep: GEMM1(n+1)'s 19 matmuls are
        # emitted BEFORE neuron n's transposes/GEMM2/GEMM3, so while the ACT
        # engine runs gelu1(n+1) the PE is never head-of-line blocked — it
        # always has a full GEMM1 of independent work queued between any
        # gelu and the transpose that consumes it.  DMA triggers are issued
        # in exact byte-consumption order, one neuron ahead:
        #   w1(n+2) | w2(n+1) w3(n+1)
        w1ts, w2ts, w3ts, h1s = {}, {}, {}, {}

        def emit_rest(n):
            h1T = transpose4(h1s[n])
            h2 = gemm2(n, w2ts[n], h1T)
            h2T = transpose4(h2)
            gemm3(n, w3ts[n], h2T)

        w1ts[0] = dma_w1(0)
        w2ts[0] = dma_w2(0)
        w3ts[0] = dma_w3(0)
        w1ts[1] = dma_w1(1)
        h1s[0] = gemm1(0, w1ts[0])
        for n in range(NPC):
            if n + 2 < NPC:
                w1ts[n + 2] = dma_w1(n + 2)
            if n + 1 < NPC:
                w2ts[n + 1] = dma_w2(n + 1)
                w3ts[n + 1] = dma_w3(n + 1)
                h1s[n + 1] = gemm1(n + 1, w1ts[n + 1])
            emit_rest(n)
            if n > 0:
                emit_B(n - 1)
        emit_B(NPC - 1)

    nc.compile()
    return nc


def _get_program():
    if "nc" not in _CACHE:
        _CACHE["nc"] = _build_program()
    return _CACHE["nc"]


def _prep_in_maps(input_embedding, pre_activations, Wp, bp, W1, b1, W2, b2, W3,
                  b3, gamma, beta, tick):
    emb = np.asarray(input_embedding, dtype=np.float32)
    hist = np.asarray(pre_activations, dtype=np.float32)
    Wp = np.asarray(Wp, dtype=np.float32)
    bp = np.asarray(bp, dtype=np.float32)
    W1 = np.asarray(W1, dtype=np.float32)
    b1 = np.asarray(b1, dtype=np.float32)
    W2 = np.asarray(W2, dtype=np.float32)
    b2 = np.asarray(b2, dtype=np.float32)
    W3 = np.asarray(W3, dtype=np.float32)
    b3 = np.asarray(b3, dtype=np.float32)
    gamma = np.asarray(gamma, dtype=np.float32)
    beta = np.asarray(beta, dtype=np.float32)

    # oscillator modulation folded into gamma/beta
    i = np.arange(N_NEURONS, dtype=np.float64)
    freq = FMIN * (FMAX / FMIN) ** (i / (N_NEURONS - 1))
    phase = np.mod(i * 2.3571, 2.0 * math.pi)
    t = float(np.asarray(tick)) * TICK_INTERVAL
    mod = (1.0 + 0.5 * np.sin(2.0 * math.pi * freq * t + phase)).astype(np.float32)
    gm = (gamma * mod[:, None]).astype(np.float32)
    bm = (beta * mod[:, None]).astype(np.float32)

    histd = np.ascontiguousarray(hist.reshape(16, 128))
    bpd = np.ascontiguousarray(bp.reshape(2, 128).T)
    eyed = np.eye(32, dtype=np.float32)

    # weight layout: (n, p, k_chunk, hid) so each supertile DMA reads one
    # contiguous run per partition; bf16 in HBM halves the DMA traffic
    W1r = np.ascontiguousarray(
        W1.reshape(N_NEURONS, KC1, 128, HID).transpose(0, 2, 1, 3)).astype(
            ml_dtypes.bfloat16)
    W2r = np.ascontiguousarray(
        W2.reshape(N_NEURONS, KC2, 128, HID).transpose(0, 2, 1, 3)).astype(
            ml_dtypes.bfloat16)
    W3r = np.ascontiguousarray(
        W3.reshape(N_NEURONS, KC2, 128, D).transpose(0, 2, 1, 3)).astype(
            ml_dtypes.bfloat16)

    # one-hot selector: sel8[k, n*B + j] = (k == n), broadcasts bvec row n
    # across the batch partitions via a K=8 matmul
    sel8 = np.zeros((NPC, NPC * B), dtype=np.float32)
    for n in range(NPC):
        sel8[n, n * B:(n + 1) * B] = 1.0

    in_maps = []
    for c in range(N_CORES):
        s = slice(c * NPC, (c + 1) * NPC)
        bvec = np.concatenate([b1[s], b2[s], b3[s], gm[s], bm[s]], axis=1)
        in_maps.append({
            "emb": emb,
            "wp": Wp,
            "bpd": bpd,
            "histd": histd,
            "eyed": eyed,
            "w1d": W1r[s],
            "w2d": W2r[s],
            "w3d": W3r[s],
            "bvecd": np.ascontiguousarray(bvec),
            "sel8d": sel8,
        })
    return in_maps


def run(inputs, trace=False):
    nc = _get_program()
    in_maps = _prep_in_maps(**inputs)
    br = run_bass_kernel_spmd(nc, in_maps, core_ids=list(range(N_CORES)),
                              trace=trace)
    out = np.concatenate([r["out"] for r in br.results], axis=1)
    return np.ascontiguousarray(out, dtype=np.float32), br


def kernel(**inputs) -> np.ndarray:
    out, _ = run(inputs, trace=False)
    return out

